# revision 25
# baseline (speedup 1.0000x reference)
"""Trainium2 Bass kernel for 16-head attention (B=4, S=2048, D=1024).

Sharding: 8 cores = 4 batches x 2 head-groups. Core c handles batch c//2,
heads (c%2)*8 .. +8. Each core computes a partial projection output
[S, D]; the host sums the two head-group partials per batch and adds
b_proj. No collectives.

Per-core layout trick: host feeds x[b] transposed (xT [D, S]), so the QKV
matmuls produce Q^T / K^T in [qkv-col, seq] layout directly, scores are
computed transposed ([sk, sq]) and softmax is done without max-subtraction
(inputs are bounded; exp stays well inside fp32/bf16 range). V is
ones-augmented so the attn@V matmul also yields softmax row-sums for free;
normalization uses a DVE reciprocal + a K=1 outer-product matmul to
broadcast the per-column scale across partitions. The normalize chain for
iteration i is emitted after iteration i+1's matmuls so the PE never waits
on the reciprocal. One unified PSUM pool lets QKV / attention / projection
matmuls interleave freely.
"""

import sys
import os

sys.path.insert(0, "/opt/trn_rl_repo")

import numpy as np
import ml_dtypes

BF = ml_dtypes.bfloat16

DIM = 1024
N_HEADS = 16
HD = 64
B = 4
S = 2048
HPC = 8          # heads per core
GC = HPC * HD    # 512 columns per head-group
N_CORES = 8
SCALE = HD ** -0.5

_CACHE = {}


def _build_bass():
    import concourse.bass as bass
    import concourse.mybir as mybir
    import concourse.tile as tile
    from concourse import bacc

    f32 = mybir.dt.float32
    bf16 = mybir.dt.bfloat16
    EXP = mybir.ActivationFunctionType.Exp

    nc = bacc.Bacc("TRN2", target_bir_lowering=False, debug=False,
                   num_devices=N_CORES)

    xT = nc.dram_tensor("xT", [DIM, S], bf16, kind="ExternalInput").ap()
    wq = nc.dram_tensor("wq", [DIM, GC], bf16, kind="ExternalInput").ap()
    wk = nc.dram_tensor("wk", [DIM, GC], bf16, kind="ExternalInput").ap()
    wv = nc.dram_tensor("wv", [DIM, GC], bf16, kind="ExternalInput").ap()
    wp = nc.dram_tensor("wp", [GC, DIM], bf16, kind="ExternalInput").ap()
    # q/k biases pre-broadcast on host: [128, m-tile*1024], each m block
    # holds the per-partition bias value replicated over 2x512 columns
    bq = nc.dram_tensor("bq", [128, 4096], f32, kind="ExternalInput").ap()
    bk = nc.dram_tensor("bk", [128, 4096], f32, kind="ExternalInput").ap()
    bvb = nc.dram_tensor("bvb", [128, GC], f32, kind="ExternalInput").ap()
    out = nc.dram_tensor("out", [S, DIM], f32, kind="ExternalOutput").ap()

    KD = DIM // 128   # 8 k-tiles over D
    NQ = GC // 128    # 4 tiles over the 512 head-group columns
    NS = S // 512     # 4 seq chunks of 512
    ST = S // 128     # 16 seq tiles of 128

    with tile.TileContext(nc) as tc:
        with tc.tile_pool(name="const", bufs=1) as cp:
            # interleave input DMAs so the first matmuls' operands land first
            xTs, wqs, wks, wvs = [], [], [], []
            for k in range(KD):
                for lst, src, nm in ((xTs, xT, "x"), (wqs, wq, "q"),
                                     (wks, wk, "k"), (wvs, wv, "v")):
                    w = S if nm == "x" else GC
                    t = cp.tile([128, w], bf16, name=f"w{nm}s{k}")
                    nc.sync.dma_start(t[:], src[k * 128:(k + 1) * 128, :])
                    lst.append(t)
            wps = []
            for k in range(NQ):
                t = cp.tile([128, DIM], bf16, name=f"wps{k}")
                nc.sync.dma_start(t[:], wp[k * 128:(k + 1) * 128, :])
                wps.append(t)
            bq_sb = cp.tile([128, 4096], f32, name="bq_sb")
            nc.sync.dma_start(bq_sb[:], bq[:, :])
            bk_sb = cp.tile([128, 4096], f32, name="bk_sb")
            nc.sync.dma_start(bk_sb[:], bk[:, :])
            bvb_sb = cp.tile([128, GC], f32, name="bvb_sb")
            nc.sync.dma_start(bvb_sb[:], bvb[:, :])
            ones_sb = cp.tile([128, 64], bf16, name="ones_sb")
            nc.any.memset(ones_sb[:], 1.0)

            QT = [cp.tile([128, S], bf16, name=f"QT{m}") for m in range(NQ)]
            KT = [cp.tile([128, S], bf16, name=f"KT{m}") for m in range(NQ)]
            # V tiles: per head 65 cols (64 data + trailing ones column)
            Vt = [cp.tile([128, HPC * 65], bf16, name=f"Vt{s}")
                  for s in range(ST)]
            OT = [cp.tile([128, S], bf16, name=f"OT{m}") for m in range(NQ)]

            for s in range(ST):
                ones_cols = Vt[s][:, :].rearrange(
                    "p (h c) -> p h c", c=65)[:, :, 64:65]
                nc.any.memset(ones_cols, 1.0)

            # one psum pool for the whole kernel: tag "s" = 3x 2-bank slots
            # (scores / qkv / proj / pb), tag "o" = 2x 1-bank accumulators
            with tc.tile_pool(name="ps", bufs=1, space="PSUM") as psp, \
                 tc.tile_pool(name="pbuf", bufs=6) as pbufp, \
                 tc.tile_pool(name="un", bufs=4) as unp, \
                 tc.tile_pool(name="rr", bufs=4) as rrp, \
                 tc.tile_pool(name="stg", bufs=3) as stgp:

                def ps_s(name):
                    return psp.tile([128, 1024], f32, tag="s", bufs=3,
                                    name=name)

                # ---- QKV projections ----
                for dst, ws, bias in ((QT, wqs, bq_sb), (KT, wks, bk_sb)):
                    for m in range(NQ):
                        for n2 in range(NS // 2):
                            ps = ps_s(f"qk{m}{n2}{id(ws) % 97}")
                            for k in range(KD):
                                for h in range(2):
                                    nc.tensor.matmul(
                                        ps[:, h * 512:(h + 1) * 512],
                                        lhsT=ws[k][:, m * 128:(m + 1) * 128],
                                        rhs=xTs[k][:, (n2 * 2 + h) * 512:
                                                   (n2 * 2 + h + 1) * 512],
                                        start=(k == 0), stop=(k == KD - 1))
                            nc.vector.tensor_add(
                                dst[m][:, n2 * 1024:(n2 + 1) * 1024], ps[:],
                                bias[:, m * 1024:(m + 1) * 1024])
                for s2 in range(ST // 2):
                    ps = ps_s(f"v{s2}")
                    for k in range(KD):
                        for h in range(2):
                            st = (s2 * 2 + h) * 128
                            nc.tensor.matmul(
                                ps[:, h * 512:(h + 1) * 512],
                                lhsT=xTs[k][:, st:st + 128],
                                rhs=wvs[k][:, :],
                                start=(k == 0), stop=(k == KD - 1))
                    for h in range(2):
                        src3 = ps[:, h * 512:(h + 1) * 512].rearrange(
                            "p (g c) -> p g c", c=64)
                        bv3 = bvb_sb[:].rearrange("p (g c) -> p g c", c=64)
                        dst3 = Vt[s2 * 2 + h][:, :].rearrange(
                            "p (g c) -> p g c", c=65)[:, :, 0:64]
                        nc.vector.tensor_add(dst3, src3, bv3)

                # ---- attention; normalize deferred by one iteration ----
                pending = None

                def emit_proj_chunk(n):
                    # proj m-tiles reading OT columns of sq chunk n
                    for m in range(4 * n, 4 * n + 4):
                        ps = ps_s(f"pj{m}")
                        for k in range(NQ):
                            for h in range(2):
                                nc.tensor.matmul(
                                    ps[:, h * 512:(h + 1) * 512],
                                    lhsT=OT[k][:, m * 128:(m + 1) * 128],
                                    rhs=wps[k][:, h * 512:(h + 1) * 512],
                                    start=(k == 0), stop=(k == NQ - 1))
                        ob = stgp.tile([128, 1024], f32, tag="ob",
                                       name=f"ob{m}")
                        nc.vector.tensor_copy(ob[:], ps[:])
                        nc.sync.dma_start(out[m * 128:(m + 1) * 128, :],
                                          ob[:])

                def emit_normalize(p):
                    hp, n, us = p
                    sq = slice(n * 512, (n + 1) * 512)
                    for half, u in ((0, us[0]), (1, us[1])):
                        r = rrp.tile([128, 512], bf16, tag="r",
                                     name=f"r{hp}{n}{half}")
                        with nc.allow_low_precision(
                                reason="bf16 softmax denom matches bf16 "
                                       "matmul precision"):
                            nc.vector.reciprocal(r[64:65, :], u[64:65, :])
                        pb = ps_s(f"pb{hp}{n}{half}")
                        nc.tensor.matmul(pb[0:64, 0:512],
                                         lhsT=ones_sb[64:65, 0:64],
                                         rhs=r[64:65, :],
                                         start=True, stop=True)
                        if half == 0:
                            nc.vector.tensor_mul(
                                OT[hp][0:64, sq], u[0:64, :],
                                pb[0:64, 0:512])
                        else:
                            stB = stgp.tile([64, 512], bf16, tag="st",
                                            name=f"stB{hp}{n}")
                            nc.vector.tensor_mul(stB[:], u[0:64, :],
                                                 pb[0:64, 0:512])
                            nc.sync.dma_start(OT[hp][64:128, sq], stB[:])
                    if hp == NQ - 1:
                        emit_proj_chunk(n)

                for n in range(NS):
                    sq = slice(n * 512, (n + 1) * 512)
                    for hp in range(NQ):
                        oA = psp.tile([128, 512], f32, tag="o", bufs=2,
                                      name=f"oA{hp}{n}")
                        oB = psp.tile([128, 512], f32, tag="o", bufs=2,
                                      name=f"oB{hp}{n}")
                        for j in range(ST):
                            sk = slice(j * 128, (j + 1) * 128)
                            # both heads' scores in one 2-bank tile; the two
                            # K=64 matmuls row-tile and overlap in the PE
                            sS = ps_s(f"sS{hp}{n}{j}")
                            nc.tensor.matmul(
                                sS[:, 0:512], lhsT=KT[hp][0:64, sk],
                                rhs=QT[hp][0:64, sq],
                                start=True, stop=True)
                            nc.tensor.matmul(
                                sS[:, 512:1024], lhsT=KT[hp][64:128, sk],
                                rhs=QT[hp][64:128, sq],
                                start=True, stop=True)
                            pT = pbufp.tile([128, 1024], bf16, tag="p",
                                            name=f"pT{hp}{n}{j}")
                            nc.scalar.activation(pT[:], sS[:], EXP,
                                                 scale=SCALE)
                            ha = hp * 2
                            nc.tensor.matmul(
                                oA[0:65, :],
                                lhsT=Vt[j][:, ha * 65:ha * 65 + 65],
                                rhs=pT[:, 0:512],
                                start=(j == 0), stop=(j == ST - 1))
                            nc.tensor.matmul(
                                oB[0:65, :],
                                lhsT=Vt[j][:, ha * 65 + 65:ha * 65 + 130],
                                rhs=pT[:, 512:1024],
                                start=(j == 0), stop=(j == ST - 1))
                        # evacuate psum accumulators to SBUF right away
                        us = []
                        for half, oPS in ((0, oA), (1, oB)):
                            u = unp.tile([128, 512], f32, tag="u",
                                         name=f"u{hp}{n}{half}")
                            nc.vector.tensor_copy(u[0:65, :], oPS[0:65, :])
                            us.append(u)
                        if pending is not None:
                            emit_normalize(pending)
                        pending = (hp, n, us)
                emit_normalize(pending)
    nc.compile()
    return nc


def _get_nc():
    if "nc" not in _CACHE:
        _CACHE["nc"] = _build_bass()
    return _CACHE["nc"]


def _in_maps(x, w_qkv, b_qkv, w_proj, b_proj):
    x = np.asarray(x, np.float32)
    w_qkv = np.asarray(w_qkv, np.float32)
    b_qkv = np.asarray(b_qkv, np.float32)
    w_proj = np.asarray(w_proj, np.float32)

    def bias_bcast(b512):
        # [128, 4096]: m-tile blocks of 1024 cols, value per partition
        col = b512.reshape(4, 128).T[:, :, None]            # [128, 4, 1]
        return np.ascontiguousarray(
            np.broadcast_to(col, (128, 4, 1024)).reshape(128, 4096))

    maps = []
    for c in range(N_CORES):
        b, g = divmod(c, 2)
        cols = slice(g * GC, (g + 1) * GC)
        wqs = w_qkv[:, 0 * DIM:1 * DIM][:, cols]
        wks = w_qkv[:, 1 * DIM:2 * DIM][:, cols]
        wvs = w_qkv[:, 2 * DIM:3 * DIM][:, cols]
        bqs = b_qkv[0 * DIM:1 * DIM][cols]
        bks = b_qkv[1 * DIM:2 * DIM][cols]
        bvs = b_qkv[2 * DIM:3 * DIM][cols]
        rows = slice(g * GC, (g + 1) * GC)
        maps.append({
            "xT": np.ascontiguousarray(x[b].T).astype(BF),
            "wq": wqs.astype(BF),
            "wk": wks.astype(BF),
            "wv": wvs.astype(BF),
            "wp": w_proj[rows, :].astype(BF),
            "bq": bias_bcast(bqs),
            "bk": bias_bcast(bks),
            "bvb": np.broadcast_to(bvs, (128, GC)).copy(),
        })
    return maps


def kernel(x, w_qkv, b_qkv, w_proj, b_proj, _trace=False):
    from concourse import bass_utils
    nc = _get_nc()
    maps = _in_maps(x, w_qkv, b_qkv, w_proj, b_proj)
    res = bass_utils.run_bass_kernel_spmd(nc, maps,
                                          core_ids=list(range(N_CORES)),
                                          trace=_trace)
    _CACHE["last_result"] = res
    b_proj = np.asarray(b_proj, np.float32)
    outs = np.empty((B, S, DIM), np.float32)
    for b in range(B):
        outs[b] = (res.results[2 * b]["out"] + res.results[2 * b + 1]["out"]
                   + b_proj)
    return outs


# revision 28
# speedup vs baseline: 1.0547x; 1.0547x over previous
"""Trainium2 Bass kernel for 16-head attention (B=4, S=2048, D=1024).

Sharding: 8 cores = 4 batches x 2 head-groups. Core c handles batch c//2,
heads (c%2)*8 .. +8. Each core computes a partial projection output
[S, D]; the host sums the two head-group partials per batch and adds
b_proj. No collectives.

Per-core layout trick: host feeds x[b] transposed (xT [D, S]), so the QKV
matmuls produce Q^T / K^T in [qkv-col, seq] layout directly, scores are
computed transposed ([sk, sq]) and softmax is done without max-subtraction
(inputs are bounded; exp stays well inside fp32/bf16 range). V is
ones-augmented so the attn@V matmul also yields softmax row-sums for free;
normalization uses a DVE reciprocal + a K=1 outer-product matmul to
broadcast the per-column scale across partitions. The normalize chain for
iteration i is emitted after iteration i+1's matmuls so the PE never waits
on the reciprocal. One unified PSUM pool lets QKV / attention / projection
matmuls interleave freely.
"""

import sys
import os

sys.path.insert(0, "/opt/trn_rl_repo")

import numpy as np
import ml_dtypes

BF = ml_dtypes.bfloat16

DIM = 1024
N_HEADS = 16
HD = 64
B = 4
S = 2048
HPC = 8          # heads per core
GC = HPC * HD    # 512 columns per head-group
N_CORES = 8
SCALE = HD ** -0.5

_CACHE = {}


def _build_bass():
    import concourse.bass as bass
    import concourse.mybir as mybir
    import concourse.tile as tile
    from concourse import bacc

    f32 = mybir.dt.float32
    bf16 = mybir.dt.bfloat16
    EXP = mybir.ActivationFunctionType.Exp

    nc = bacc.Bacc("TRN2", target_bir_lowering=False, debug=False,
                   num_devices=N_CORES)

    xT = nc.dram_tensor("xT", [DIM, S], bf16, kind="ExternalInput").ap()
    wq = nc.dram_tensor("wq", [DIM, GC], bf16, kind="ExternalInput").ap()
    wk = nc.dram_tensor("wk", [DIM, GC], bf16, kind="ExternalInput").ap()
    wv = nc.dram_tensor("wv", [DIM, GC], bf16, kind="ExternalInput").ap()
    wp = nc.dram_tensor("wp", [GC, DIM], bf16, kind="ExternalInput").ap()
    # q/k biases pre-broadcast on host: [128, m-tile*1024], each m block
    # holds the per-partition bias value replicated over 2x512 columns
    bq = nc.dram_tensor("bq", [128, 4096], f32, kind="ExternalInput").ap()
    bk = nc.dram_tensor("bk", [128, 4096], f32, kind="ExternalInput").ap()
    bvb = nc.dram_tensor("bvb", [128, GC], f32, kind="ExternalInput").ap()
    out = nc.dram_tensor("out", [S, DIM], f32, kind="ExternalOutput").ap()

    KD = DIM // 128   # 8 k-tiles over D
    NQ = GC // 128    # 4 tiles over the 512 head-group columns
    NS = S // 512     # 4 seq chunks of 512
    ST = S // 128     # 16 seq tiles of 128

    with tile.TileContext(nc) as tc:
        with tc.tile_pool(name="const", bufs=1) as cp:
            # interleave input DMAs so the first matmuls' operands land first
            xTs, wqs, wks, wvs = [], [], [], []
            for k in range(KD):
                for lst, src, nm in ((xTs, xT, "x"), (wqs, wq, "q"),
                                     (wks, wk, "k"), (wvs, wv, "v")):
                    w = S if nm == "x" else GC
                    t = cp.tile([128, w], bf16, name=f"w{nm}s{k}")
                    nc.sync.dma_start(t[:], src[k * 128:(k + 1) * 128, :])
                    lst.append(t)
            wps = []
            for k in range(NQ):
                t = cp.tile([128, DIM], bf16, name=f"wps{k}")
                nc.sync.dma_start(t[:], wp[k * 128:(k + 1) * 128, :])
                wps.append(t)
            bq_sb = cp.tile([128, 4096], f32, name="bq_sb")
            nc.sync.dma_start(bq_sb[:], bq[:, :])
            bk_sb = cp.tile([128, 4096], f32, name="bk_sb")
            nc.sync.dma_start(bk_sb[:], bk[:, :])
            bvb_sb = cp.tile([128, GC], f32, name="bvb_sb")
            nc.sync.dma_start(bvb_sb[:], bvb[:, :])
            ones_sb = cp.tile([128, 64], bf16, name="ones_sb")
            nc.any.memset(ones_sb[:], 1.0)

            QT = [cp.tile([128, S], bf16, name=f"QT{m}") for m in range(NQ)]
            KT = [cp.tile([128, S], bf16, name=f"KT{m}") for m in range(NQ)]
            # V tiles: per head 65 cols (64 data + trailing ones column)
            Vt = [cp.tile([128, HPC * 65], bf16, name=f"Vt{s}")
                  for s in range(ST)]
            OT = [cp.tile([128, S], bf16, name=f"OT{m}") for m in range(NQ)]

            for s in range(ST):
                ones_cols = Vt[s][:, :].rearrange(
                    "p (h c) -> p h c", c=65)[:, :, 64:65]
                nc.any.memset(ones_cols, 1.0)

            # one psum pool for the whole kernel: tag "s" = 3x 2-bank slots
            # (scores / qkv / proj / pb), tag "o" = 2x 1-bank accumulators
            with tc.tile_pool(name="ps", bufs=1, space="PSUM") as psp, \
                 tc.tile_pool(name="pbuf", bufs=6) as pbufp, \
                 tc.tile_pool(name="un", bufs=4) as unp, \
                 tc.tile_pool(name="rr", bufs=4) as rrp, \
                 tc.tile_pool(name="stg", bufs=3) as stgp:

                def ps_s(name):
                    return psp.tile([128, 1024], f32, tag="s", bufs=3,
                                    name=name)

                # ---- QKV projections ----
                for dst, ws, bias in ((QT, wqs, bq_sb), (KT, wks, bk_sb)):
                    for m in range(NQ):
                        for n2 in range(NS // 2):
                            ps = ps_s(f"qk{m}{n2}{id(ws) % 97}")
                            for k in range(KD):
                                for h in range(2):
                                    nc.tensor.matmul(
                                        ps[:, h * 512:(h + 1) * 512],
                                        lhsT=ws[k][:, m * 128:(m + 1) * 128],
                                        rhs=xTs[k][:, (n2 * 2 + h) * 512:
                                                   (n2 * 2 + h + 1) * 512],
                                        start=(k == 0), stop=(k == KD - 1))
                            nc.vector.tensor_add(
                                dst[m][:, n2 * 1024:(n2 + 1) * 1024], ps[:],
                                bias[:, m * 1024:(m + 1) * 1024])
                for s2 in range(ST // 2):
                    ps = ps_s(f"v{s2}")
                    for k in range(KD):
                        for h in range(2):
                            st = (s2 * 2 + h) * 128
                            nc.tensor.matmul(
                                ps[:, h * 512:(h + 1) * 512],
                                lhsT=xTs[k][:, st:st + 128],
                                rhs=wvs[k][:, :],
                                start=(k == 0), stop=(k == KD - 1))
                    for h in range(2):
                        src3 = ps[:, h * 512:(h + 1) * 512].rearrange(
                            "p (g c) -> p g c", c=64)
                        bv3 = bvb_sb[:].rearrange("p (g c) -> p g c", c=64)
                        dst3 = Vt[s2 * 2 + h][:, :].rearrange(
                            "p (g c) -> p g c", c=65)[:, :, 0:64]
                        nc.vector.tensor_add(dst3, src3, bv3)

                # ---- attention; normalize deferred by one iteration ----
                pending = None

                def emit_normalize(p):
                    hp, n, us = p
                    sq = slice(n * 512, (n + 1) * 512)
                    for half, u in ((0, us[0]), (1, us[1])):
                        r = rrp.tile([128, 512], bf16, tag="r",
                                     name=f"r{hp}{n}{half}")
                        with nc.allow_low_precision(
                                reason="bf16 softmax denom matches bf16 "
                                       "matmul precision"):
                            nc.vector.reciprocal(r[64:65, :], u[64:65, :])
                        pb = ps_s(f"pb{hp}{n}{half}")
                        nc.tensor.matmul(pb[0:64, 0:512],
                                         lhsT=ones_sb[64:65, 0:64],
                                         rhs=r[64:65, :],
                                         start=True, stop=True)
                        if half == 0:
                            nc.vector.tensor_mul(
                                OT[hp][0:64, sq], u[0:64, :],
                                pb[0:64, 0:512])
                        else:
                            stB = stgp.tile([64, 512], bf16, tag="st",
                                            name=f"stB{hp}{n}")
                            nc.vector.tensor_mul(stB[:], u[0:64, :],
                                                 pb[0:64, 0:512])
                            nc.sync.dma_start(OT[hp][64:128, sq], stB[:])

                for n in range(NS):
                    sq = slice(n * 512, (n + 1) * 512)
                    for hp in range(NQ):
                        oA = psp.tile([128, 512], f32, tag="o", bufs=2,
                                      name=f"oA{hp}{n}")
                        oB = psp.tile([128, 512], f32, tag="o", bufs=2,
                                      name=f"oB{hp}{n}")
                        for j in range(ST):
                            sk = slice(j * 128, (j + 1) * 128)
                            # both heads' scores in one 2-bank tile; the two
                            # K=64 matmuls row-tile and overlap in the PE
                            sS = ps_s(f"sS{hp}{n}{j}")
                            nc.tensor.matmul(
                                sS[:, 0:512], lhsT=KT[hp][0:64, sk],
                                rhs=QT[hp][0:64, sq],
                                start=True, stop=True)
                            nc.tensor.matmul(
                                sS[:, 512:1024], lhsT=KT[hp][64:128, sk],
                                rhs=QT[hp][64:128, sq],
                                start=True, stop=True)
                            pT = pbufp.tile([128, 1024], bf16, tag="p",
                                            name=f"pT{hp}{n}{j}")
                            nc.scalar.activation(pT[:], sS[:], EXP,
                                                 scale=SCALE)
                            ha = hp * 2
                            nc.tensor.matmul(
                                oA[0:65, :],
                                lhsT=Vt[j][:, ha * 65:ha * 65 + 65],
                                rhs=pT[:, 0:512],
                                start=(j == 0), stop=(j == ST - 1))
                            nc.tensor.matmul(
                                oB[0:65, :],
                                lhsT=Vt[j][:, ha * 65 + 65:ha * 65 + 130],
                                rhs=pT[:, 512:1024],
                                start=(j == 0), stop=(j == ST - 1))
                        # evacuate psum accumulators to SBUF right away
                        us = []
                        for half, oPS in ((0, oA), (1, oB)):
                            u = unp.tile([128, 512], f32, tag="u",
                                         name=f"u{hp}{n}{half}")
                            nc.vector.tensor_copy(u[0:65, :], oPS[0:65, :])
                            us.append(u)
                        if pending is not None:
                            emit_normalize(pending)
                        pending = (hp, n, us)
                emit_normalize(pending)

                # ---- output projection (partial over this head-group) ----
                for m in range(ST):
                    ps = ps_s(f"pj{m}")
                    for k in range(NQ):
                        for h in range(2):
                            nc.tensor.matmul(
                                ps[:, h * 512:(h + 1) * 512],
                                lhsT=OT[k][:, m * 128:(m + 1) * 128],
                                rhs=wps[k][:, h * 512:(h + 1) * 512],
                                start=(k == 0), stop=(k == NQ - 1))
                    ob = stgp.tile([128, 1024], f32, tag="ob",
                                   name=f"ob{m}")
                    nc.vector.tensor_copy(ob[:], ps[:])
                    nc.sync.dma_start(out[m * 128:(m + 1) * 128, :], ob[:])
    nc.compile()
    return nc


def _get_nc():
    if "nc" not in _CACHE:
        _CACHE["nc"] = _build_bass()
    return _CACHE["nc"]


def _in_maps(x, w_qkv, b_qkv, w_proj, b_proj):
    x = np.asarray(x, np.float32)
    w_qkv = np.asarray(w_qkv, np.float32)
    b_qkv = np.asarray(b_qkv, np.float32)
    w_proj = np.asarray(w_proj, np.float32)

    def bias_bcast(b512):
        # [128, 4096]: m-tile blocks of 1024 cols, value per partition
        col = b512.reshape(4, 128).T[:, :, None]            # [128, 4, 1]
        return np.ascontiguousarray(
            np.broadcast_to(col, (128, 4, 1024)).reshape(128, 4096))

    maps = []
    for c in range(N_CORES):
        b, g = divmod(c, 2)
        cols = slice(g * GC, (g + 1) * GC)
        wqs = w_qkv[:, 0 * DIM:1 * DIM][:, cols]
        wks = w_qkv[:, 1 * DIM:2 * DIM][:, cols]
        wvs = w_qkv[:, 2 * DIM:3 * DIM][:, cols]
        bqs = b_qkv[0 * DIM:1 * DIM][cols]
        bks = b_qkv[1 * DIM:2 * DIM][cols]
        bvs = b_qkv[2 * DIM:3 * DIM][cols]
        rows = slice(g * GC, (g + 1) * GC)
        maps.append({
            "xT": np.ascontiguousarray(x[b].T).astype(BF),
            "wq": wqs.astype(BF),
            "wk": wks.astype(BF),
            "wv": wvs.astype(BF),
            "wp": w_proj[rows, :].astype(BF),
            "bq": bias_bcast(bqs),
            "bk": bias_bcast(bks),
            "bvb": np.broadcast_to(bvs, (128, GC)).copy(),
        })
    return maps


def kernel(x, w_qkv, b_qkv, w_proj, b_proj, _trace=False):
    from concourse import bass_utils
    nc = _get_nc()
    maps = _in_maps(x, w_qkv, b_qkv, w_proj, b_proj)
    res = bass_utils.run_bass_kernel_spmd(nc, maps,
                                          core_ids=list(range(N_CORES)),
                                          trace=_trace)
    _CACHE["last_result"] = res
    b_proj = np.asarray(b_proj, np.float32)
    outs = np.empty((B, S, DIM), np.float32)
    for b in range(B):
        outs[b] = (res.results[2 * b]["out"] + res.results[2 * b + 1]["out"]
                   + b_proj)
    return outs
